# revision 6
# baseline (speedup 1.0000x reference)
"""Bahdanau attention on 8 Trainium2 NeuronCores.

Data-parallel over batch: each core handles 8 of the 64 batches.
Per batch b (shapes per core; S=F=U=1024):
  projT[u,s]  = sum_f W1[f,u] * feat[s,f]          (PE, bf16, out [u,s] in PSUM)
  scoreT      = tanh(projT + bias[u])              (ACT, per-partition bias)
  logits[s]   = sum_u scoreT[u,s] * V[u]           (PE, V stationary)
  w[s]        = softmax(logits)  (no max-sub: |logits| <= ||V||_1, exp safe)
  ctx[f]      = sum_s w[s] * feat[s,f]             (PE, f32r, w column stationary)

features are cast to bf16 during the HBM load (SWDGE), kept in natural [s,f]
layout for the ctx pass, and transposed to [f,s] via the DMA xbar for the big
matmul.  bias[u] = hidden@W2 + W2_b + W1_b is computed once per core for all
8 local batches.  V_b provably cancels in softmax and never affects outputs.
"""

import numpy as np

import concourse.bacc as bacc
import concourse.tile as tile
import concourse.mybir as mybir
from concourse.bass_utils import run_bass_kernel_spmd
from concourse.bass_interp import get_hw_module

dt = mybir.dt
AF = mybir.ActivationFunctionType

N_CORES = 8
B, S, F, U = 64, 1024, 1024, 1024
BL = B // N_CORES          # batches per core
P = 128                    # partitions
NT = 1024 // P             # 8 tiles along any 1024 dim
H2 = 512                   # psum half (fp32 bank = 512 els)




def _build(n_batches=BL):
    nc = bacc.Bacc("TRN2", target_bir_lowering=False, debug=False, num_devices=1)

    feat_d = nc.dram_tensor("features", [n_batches, S, F], dt.float32, kind="ExternalInput").ap()
    hid_d = nc.dram_tensor("hidden", [n_batches, U], dt.float32, kind="ExternalInput").ap()
    w1_d = nc.dram_tensor("W1_w", [F, U], dt.float32, kind="ExternalInput").ap()
    w1b_d = nc.dram_tensor("W1_b", [U], dt.float32, kind="ExternalInput").ap()
    w2_d = nc.dram_tensor("W2_w", [U, U], dt.float32, kind="ExternalInput").ap()
    w2b_d = nc.dram_tensor("W2_b", [U], dt.float32, kind="ExternalInput").ap()
    v_d = nc.dram_tensor("V_w", [U, 1], dt.float32, kind="ExternalInput").ap()
    id8_d = nc.dram_tensor("ident8", [n_batches, n_batches], dt.float32, kind="ExternalInput").ap()
    id1_d = nc.dram_tensor("ident1", [1, 1], dt.float32, kind="ExternalInput").ap()

    ctx_d = nc.dram_tensor("ctx", [n_batches, F], dt.float32, kind="ExternalOutput").ap()
    attn_d = nc.dram_tensor("attn", [n_batches, S], dt.float32, kind="ExternalOutput").ap()

    with tile.TileContext(nc) as tc:
        # ---------------- persistent weights / constants ----------------
        with tc.tile_pool(name="wpool", bufs=1) as wp:
            # W1 as bf16, [p][ft][u] with f = ft*128 + p (contiguous f chunks)
            w1_sb = wp.tile([P, NT * U], dt.bfloat16)
            w1_v = w1_sb[:].rearrange("p (t u) -> p t u", t=NT)
            nc.gpsimd.dma_start(w1_v, w1_d.rearrange("(t p) u -> p t u", p=P))

            # V as bf16 columns: col ui holds V[ui*128 : (ui+1)*128]
            v_sb = wp.tile([P, NT], dt.bfloat16)
            nc.gpsimd.dma_start(v_sb[:], v_d.rearrange("(t p) o -> p (t o)", p=P))

            # W1_b + W2_b as columns [p, ui]
            b1_sb = wp.tile([P, NT], dt.float32)
            nc.sync.dma_start(b1_sb[:], w1b_d.rearrange("(t p) -> p t", p=P))
            b2_sb = wp.tile([P, NT], dt.float32)
            nc.sync.dma_start(b2_sb[:], w2b_d.rearrange("(t p) -> p t", p=P))
            wb_sb = wp.tile([P, NT], dt.float32)
            nc.vector.tensor_add(wb_sb[:], b1_sb[:], b2_sb[:])

            id1_sb = wp.tile([1, 1], dt.float32)
            nc.sync.dma_start(id1_sb[:], id1_d[:])

            # bias[p, ui*BL + b] = (hidden @ W2 + W2_b + W1_b)[b, ui*128+p]
            bias_sb = wp.tile([P, NT * n_batches], dt.float32)

            # ---- preamble: proj_h = hidden @ W2 (transposed) ----
            with tc.tile_pool(name="prepool", bufs=1) as pp, \
                 tc.tile_pool(name="preps", bufs=2, space="PSUM") as pps:
                w2_sb = pp.tile([P, NT * U], dt.float32)
                w2_v = w2_sb[:].rearrange("p (t u) -> p t u", t=NT)
                nc.sync.dma_start(w2_v, w2_d.rearrange("(t p) u -> p t u", p=P))

                hid_sb = pp.tile([n_batches, U], dt.float32)
                nc.sync.dma_start(hid_sb[:], hid_d[:])
                id8_sb = pp.tile([n_batches, n_batches], dt.float32)
                nc.sync.dma_start(id8_sb[:], id8_d[:])

                # hiddenT [h, b] via PE transpose, chunk j: [bl,128] -> [128,bl]
                ht_ps = pps.tile([P, NT * n_batches], dt.float32)
                for j in range(NT):
                    nc.tensor.matmul(
                        ht_ps[:, j * n_batches:(j + 1) * n_batches],
                        hid_sb[:, j * P:(j + 1) * P],
                        id8_sb[:],
                        is_transpose=True,
                    )
                ht_sb = pp.tile([P, NT * n_batches], dt.float32)
                nc.vector.tensor_copy(ht_sb[:], ht_ps[:])
                ht_v = ht_sb[:].rearrange("p (t b) -> p t b", t=NT)

                for ui in range(NT):
                    ph = pps.tile([P, n_batches], dt.float32, tag="ph")
                    for j in range(NT):
                        nc.tensor.matmul(
                            ph[:],
                            w2_v[:, j, ui * P:(ui + 1) * P],
                            ht_v[:, j, :],
                            start=(j == 0),
                            stop=(j == NT - 1),
                        )
                    nc.vector.tensor_scalar_add(
                        bias_sb[:, ui * n_batches:(ui + 1) * n_batches],
                        ph[:],
                        wb_sb[:, ui:ui + 1],
                    )

            # ---------------- main per-batch pipeline ----------------
            with tc.tile_pool(name="featbf", bufs=2) as fbp, \
                 tc.tile_pool(name="featT", bufs=2) as ftp, \
                 tc.tile_pool(name="score", bufs=2) as scp, \
                 tc.tile_pool(name="small", bufs=2) as sm, \
                 tc.tile_pool(name="pspf", bufs=3, space="PSUM") as pspf, \
                 tc.tile_pool(name="pslg", bufs=2, space="PSUM") as pslg, \
                 tc.tile_pool(name="pswt", bufs=2, space="PSUM") as pswt, \
                 tc.tile_pool(name="psctx", bufs=1, space="PSUM") as psctx:

                for b in range(n_batches):
                    # natural bf16 features [p][st][f], s = st*128 + p
                    # (cast fp32 -> bf16 during the DMA, SWDGE)
                    featbf = fbp.tile([P, NT * F], dt.bfloat16, tag="featbf")
                    featbf_v = featbf[:].rearrange("p (t f) -> p t f", t=NT)
                    nc.gpsimd.dma_start(featbf_v, feat_d[b].rearrange("(t p) f -> p t f", p=P))

                    # xbar transpose -> featT [p][ft][s], f = ft*128 + p
                    feattr = ftp.tile([P, NT * S], dt.bfloat16, tag="feattr")
                    feattr_v = feattr[:].rearrange("p (t s) -> p t s", t=NT)
                    for st in range(NT):
                        nc.sync.dma_start_transpose(
                            feattr_v[:, :, st * P:(st + 1) * P],
                            featbf[:, st * F:(st + 1) * F],
                        )

                    # big matmul + tanh -> scoreT [p][ui][s] bf16
                    score = scp.tile([P, NT * S], dt.bfloat16, tag="score")
                    score_v = score[:].rearrange("p (t s) -> p t s", t=NT)
                    for ui in range(NT):
                        pf0 = pspf.tile([P, H2], dt.float32, tag="pf")
                        pf1 = pspf.tile([P, H2], dt.float32, tag="pf")
                        for ft in range(NT):
                            for h, pf in ((0, pf0), (1, pf1)):
                                nc.tensor.matmul(
                                    pf[:],
                                    w1_v[:, ft, ui * P:(ui + 1) * P],
                                    feattr_v[:, ft, h * H2:(h + 1) * H2],
                                    start=(ft == 0),
                                    stop=(ft == NT - 1),
                                )
                        for h, pf in ((0, pf0), (1, pf1)):
                            nc.scalar.activation(
                                score_v[:, ui, h * H2:(h + 1) * H2],
                                pf[:],
                                AF.Tanh,
                                bias=bias_sb[:, ui * n_batches + b: ui * n_batches + b + 1],
                                scale=1.0,
                            )

                    # logits = V . scoreT  (PE), then exp + sum (ACT)
                    e_row = sm.tile([1, S], dt.float32, tag="erow")
                    zh = sm.tile([1, 2], dt.float32, tag="zh")
                    for h in range(2):
                        lg = pslg.tile([1, H2], dt.float32, tag="lg")
                        for ui in range(NT):
                            nc.tensor.matmul(
                                lg[:],
                                v_sb[:, ui:ui + 1],
                                score_v[:, ui, h * H2:(h + 1) * H2],
                                start=(ui == 0),
                                stop=(ui == NT - 1),
                            )
                        nc.scalar.activation(
                            e_row[:, h * H2:(h + 1) * H2],
                            lg[:],
                            AF.Exp,
                            accum_out=zh[:, h:h + 1],
                        )

                    z = sm.tile([1, 1], dt.float32, tag="z")
                    nc.vector.tensor_add(z[:], zh[:, 0:1], zh[:, 1:2])
                    rz = sm.tile([1, 1], dt.float32, tag="rz")
                    nc.vector.reciprocal(rz[:], z[:])

                    w_row = sm.tile([1, S], dt.float32, tag="wrow")
                    nc.vector.tensor_scalar_mul(w_row[:], e_row[:], rz[:])
                    nc.sync.dma_start(attn_d[b:b + 1, :], w_row[:])

                    # transpose w to columns [s_p, st] (PE transpose per chunk)
                    wt_sb = sm.tile([P, NT], dt.bfloat16, tag="wt")
                    for st in range(NT):
                        wt_ps = pswt.tile([P, 1], dt.float32, tag="wtp")
                        nc.tensor.matmul(
                            wt_ps[:],
                            w_row[:, st * P:(st + 1) * P],
                            id1_sb[:],
                            is_transpose=True,
                        )
                        nc.vector.tensor_copy(wt_sb[:, st:st + 1], wt_ps[:])

                    # ctx[f] = sum_s w[s] * feat[s, f]   (bf16 matmuls)
                    ctx_row = sm.tile([1, F], dt.float32, tag="ctxrow")
                    for fh in range(2):
                        cp = psctx.tile([1, H2], dt.float32, tag="cp")
                        for st in range(NT):
                            nc.tensor.matmul(
                                cp[:],
                                wt_sb[:, st:st + 1],
                                featbf_v[:, st, fh * H2:(fh + 1) * H2],
                                start=(st == 0),
                                stop=(st == NT - 1),
                            )
                        nc.scalar.copy(ctx_row[:, fh * H2:(fh + 1) * H2], cp[:])
                    nc.sync.dma_start(ctx_d[b:b + 1, :], ctx_row[:])

    nc.compile()
    nc.m = get_hw_module(nc.m)
    return nc


_NC_CACHE = {}


def _get_nc(n_batches=BL):
    if n_batches not in _NC_CACHE:
        _NC_CACHE[n_batches] = _build(n_batches)
    return _NC_CACHE[n_batches]


def kernel(features, hidden, W1_w, W1_b, W2_w, W2_b, V_w, V_b, **_ignored):
    features = np.ascontiguousarray(np.asarray(features, dtype=np.float32))
    hidden = np.ascontiguousarray(np.asarray(hidden, dtype=np.float32))
    W1_w = np.ascontiguousarray(np.asarray(W1_w, dtype=np.float32))
    W1_b = np.ascontiguousarray(np.asarray(W1_b, dtype=np.float32))
    W2_w = np.ascontiguousarray(np.asarray(W2_w, dtype=np.float32))
    W2_b = np.ascontiguousarray(np.asarray(W2_b, dtype=np.float32))
    V_w = np.ascontiguousarray(np.asarray(V_w, dtype=np.float32))
    # V_b shifts all logits equally; softmax and both outputs are invariant.

    nc = _get_nc(BL)
    id8 = np.eye(BL, dtype=np.float32)
    id1 = np.ones((1, 1), np.float32)

    in_maps = []
    for c in range(N_CORES):
        sl = slice(c * BL, (c + 1) * BL)
        in_maps.append({
            "features": features[sl],
            "hidden": hidden[sl],
            "W1_w": W1_w, "W1_b": W1_b,
            "W2_w": W2_w, "W2_b": W2_b,
            "V_w": V_w,
            "ident8": id8, "ident1": id1,
        })

    res = run_bass_kernel_spmd(nc, in_maps, list(range(N_CORES)))

    ctx = np.concatenate([r["ctx"] for r in res.results], axis=0)
    attn = np.concatenate([r["attn"] for r in res.results], axis=0)
    return ctx.astype(np.float32), attn.reshape(B, S, 1).astype(np.float32)
